# revision 28
# baseline (speedup 1.0000x reference)
"""Single-head causal attention (B=4, S=4096, E=512, D=64) on 8 trn2 cores.

Sharding: 8 cores = 4 batches x 2 query-interleave groups. Core (b, h)
computes output for batch b, query tiles {h, h+2, ..., h+30} (128 rows
each, 16 tiles = 2048 queries). Each core computes K/V for the full
sequence of its batch from x (duplicated across the batch's core pair --
no cross-core collectives).

To keep the SPMD program identical across cores, the host permutes the
key/sequence tiles per core (pair-swap for h=1) so that a core's query
tiles always sit at even SBUF tile slots and the block-causal structure
is slot-identical across cores. Exact causality inside the "diagonal
band" is applied with a data-driven 0/1 mask built on device from
per-core position vectors (qband/kband inputs).

Per-core program (Tile framework, fp32r matmuls, fp32 accumulation):
  phase B (per 512-key chunk): DMA x^T chunk, project [K|V] with a
    single M=128 matmul per contraction chunk, PE-transpose V^T blocks
    into [k,d] layout, project Q^T for the matching query chunk.
  phase C (per 512-query chunk c): for key blocks j=0..8c+7: scoresT_j =
    K_j Q^T (contraction d=64), exp via ACT (scale=1/8) into SBUF, band
    mask multiply, accumulate [V_j|1]^T expT into psum -> [attnT; denom].
    The psum [65, 512] block goes straight to HBM; the host divides by
    the denominator row during the gather.
"""

import numpy as np
from contextlib import ExitStack

import concourse.mybir as mybir
import concourse.tile as tile
from concourse import bacc
from concourse.bass_utils import run_bass_kernel_spmd
from concourse.masks import make_identity

F32 = mybir.dt.float32
F32R = mybir.dt.float32r
BF16 = mybir.dt.bfloat16
AF = mybir.ActivationFunctionType
OP = mybir.AluOpType

B, S, E, D = 4, 4096, 512, 64
P = 128
EO = E // P           # 4 contraction chunks of 128
NT = S // P           # 32 key tiles
KC = S // 512         # 8 key chunks
QC = (S // 2) // 512  # 4 query chunks per core
N_CORES = 8

_CACHE: dict = {}


def _build():
    nc = bacc.Bacc(
        "TRN2", target_bir_lowering=False, debug=False, num_devices=N_CORES
    )
    xkT = nc.dram_tensor("xkT", [E, S], BF16, kind="ExternalInput").ap()
    w3 = nc.dram_tensor("w3", [E, 3 * D], BF16, kind="ExternalInput").ap()
    bias2 = nc.dram_tensor("bias2", [P, 2], F32, kind="ExternalInput").ap()
    qband = nc.dram_tensor("qband", [P, 512], F32, kind="ExternalInput").ap()
    kband = nc.dram_tensor("kband", [P, 8], F32, kind="ExternalInput").ap()
    # rows 0:64 attnT numerator, row 64 softmax denominator
    outT = nc.dram_tensor("outT", [D + 1, S // 2], F32, kind="ExternalOutput").ap()

    with tile.TileContext(nc) as tc, ExitStack() as ctx:
        sb_const = ctx.enter_context(tc.tile_pool(name="const", bufs=1))
        sb_kv = ctx.enter_context(tc.tile_pool(name="kv", bufs=1))
        sb_xk = ctx.enter_context(tc.tile_pool(name="xk", bufs=6))
        sb_exp = ctx.enter_context(tc.tile_pool(name="exp", bufs=5))
        ps_misc = ctx.enter_context(tc.tile_pool(name="psm", bufs=2, space="PSUM"))
        ps_sc = ctx.enter_context(tc.tile_pool(name="pssc", bufs=2, space="PSUM"))
        ps_at = ctx.enter_context(tc.tile_pool(name="psat", bufs=1, space="PSUM"))
        ps_q = ctx.enter_context(tc.tile_pool(name="psq", bufs=1, space="PSUM"))

        # ---------------- constants ----------------
        w3t = sb_const.tile([P, EO, 3 * D], BF16)
        nc.sync.dma_start(
            w3t[:], w3.rearrange("(eo p) d -> p eo d", p=P)
        )
        b2 = sb_const.tile([P, 2], F32)
        nc.sync.dma_start(b2[:], bias2)
        qb = sb_const.tile([P, 512], F32)
        kb = sb_const.tile([P, 8], F32)
        onesF = sb_const.tile([P, D], F32)
        nc.gpsimd.memset(onesF[:], 1.0)
        # 0/1 causal band masks, restricted to the slots that actually
        # need masking: band pair m covers query cols [qs(m), qs(m)+w(m))
        # with qs = 128*m (m<3) else 256, w = 128 (m<3) else 256.
        bmask = sb_const.tile([P, 8, P], F32R)

        def build_masks():
            nc.sync.dma_start(qb[:], qband)
            nc.sync.dma_start(kb[:], kband)
            for jl in range(8):
                m = jl // 2
                qc0 = 128 * m
                nc.vector.tensor_tensor(
                    out=bmask[:, jl, :],
                    in0=qb[:, qc0 : qc0 + P],
                    in1=kb[:, jl : jl + 1].to_broadcast((P, P)),
                    op=OP.is_ge,
                )

        # ---------------- persistent state ----------------
        # kv_all = biased K^T
        kv_all = sb_kv.tile([D, S], F32R)
        qts = sb_kv.tile([D, S // 2], F32R)
        # V blocks in [k, d] layout plus a ones column for the denominator
        vo = sb_kv.tile([P, NT, D + 1], F32R)
        nc.vector.tensor_copy(vo[:, :, D], onesF[:, 0:NT])

        def phase_b(kc):
            xk = sb_xk.tile([P, EO, 512], BF16, tag="xk", name=f"xk{kc}")
            src = xkT[:, kc * 512 : (kc + 1) * 512].rearrange(
                "(eo p) k -> p eo k", p=P
            )
            if kc < 2:
                # per-eo DMAs so the first matmuls start at 1/4 of the load
                for eo in range(EO):
                    nc.sync.dma_start(xk[:, eo, :], src[:, eo, :])
            else:
                nc.sync.dma_start(xk[:], src)
            # K^T projection (M=64 stationary operand)
            pkv = ps_misc.tile([P, 512], F32, tag="ps", name=f"pkv{kc}")
            for eo in range(EO):
                nc.tensor.matmul(
                    pkv[0:D, :],
                    w3t[:, eo, D : 2 * D],
                    xk[:, eo, :],
                    start=(eo == 0),
                    stop=(eo == EO - 1),
                )
            nc.vector.tensor_tensor(
                out=kv_all[:, kc * 512 : (kc + 1) * 512],
                in0=pkv[0:D, :],
                in1=b2[0:D, 0:1].to_broadcast((D, 512)),
                op=OP.add,
            )
            return xk


        def phase_b_tr(kc):
            # direct V projection in [k, d] layout: the bf16 key block is
            # the stationary operand, Wv streams (N=64); V bias is applied
            # on the host after the division
            xk = xk_tiles[kc]
            pt = ps_misc.tile([P, 512], F32, tag="ps", name=f"pt{kc}")
            for bb in range(4):
                for eo in range(EO):
                    nc.tensor.matmul(
                        pt[:, bb * D : (bb + 1) * D],
                        xk[:, eo, bb * P : (bb + 1) * P],
                        w3t[:, eo, 2 * D : 3 * D],
                        start=(eo == 0),
                        stop=(eo == EO - 1),
                    )
            nc.vector.tensor_copy(
                vo[:, 4 * kc : 4 * kc + 4, 0:D],
                pt[:, 0 : 4 * D].rearrange("p (b d) -> p b d", d=D),
            )

        pq_tiles = {}

        def phase_b_q_half(c, xk_h, half):
            # Q^T half-chunk from the even local tiles of one key chunk
            if half == 0:
                pq_tiles[c] = ps_q.tile([P, 512], F32, tag="pq", name=f"pq{c}")
            pq = pq_tiles[c]
            for eo in range(EO):
                rhs = xk_h[:, eo, :].rearrange(
                    "p (t2 two x) -> p t2 two x", two=2, x=P
                )[:, :, 0, :]
                nc.tensor.matmul(
                    pq[0:D, half * 256 : (half + 1) * 256],
                    w3t[:, eo, 0:D],
                    rhs,
                    start=(eo == 0),
                    stop=(eo == EO - 1),
                )
            nc.vector.tensor_tensor(
                out=qts[:, c * 512 + half * 256 : c * 512 + (half + 1) * 256],
                in0=pq[0:D, half * 256 : (half + 1) * 256],
                in1=b2[0:D, 1:2].to_broadcast((D, 256)),
                op=OP.add,
            )

        def phase_b_q(c, xk_a, xk_b):
            phase_b_q_half(c, xk_a, 0)
            phase_b_q_half(c, xk_b, 1)

        def phase_c(c, inject=None):
            pat = ps_at.tile([D + 1, 512], F32, tag="at", name=f"at{c}")
            npair = 4 * c + 4
            # band pairs first: they depend on the same key chunks as this
            # chunk's Q projection, and ending on a mask-free full-width
            # pair shortens the critical tail
            order = list(range(4 * c, npair)) + list(range(0, 4 * c))
            for idx, p2 in enumerate(order):
                if inject and idx in inject:
                    for fn in inject[idx]:
                        fn()
                j0, j1 = 2 * p2, 2 * p2 + 1
                m = p2 - 4 * c  # band pair index, >= 0 inside the band
                qs = 0 if m < 0 else min(128 * m, 256)  # matmul region
                qe = 0 if m < 0 else 128 * m            # exp/mask/AV region
                psc = ps_sc.tile([P, 1024], F32, tag="sc", name=f"sc{c}_{p2}")
                # row-tiled pair: j0 on array rows 0:64, j1 on rows 64:128
                nc.tensor.matmul(
                    psc[:, qs:512],
                    kv_all[0:D, j0 * P : (j0 + 1) * P],
                    qts[:, c * 512 + qs : (c + 1) * 512],
                    start=True,
                    stop=True,
                    tile_position=(0, 0),
                )
                nc.tensor.matmul(
                    psc[:, 512 + qs : 1024],
                    kv_all[0:D, j1 * P : (j1 + 1) * P],
                    qts[:, c * 512 + qs : (c + 1) * 512],
                    start=True,
                    stop=True,
                )
                eT = sb_exp.tile([P, 1024], F32R, tag="eT", name=f"eT{c}_{p2}")
                psc_v = psc[:].rearrange("p (two x) -> p two x", x=512)
                eT_v = eT[:].rearrange("p (two x) -> p two x", x=512)
                nc.scalar.activation(
                    eT_v[:, :, qe:512], psc_v[:, :, qe:512], AF.Exp, scale=0.125
                )
                if m >= 0:
                    nc.vector.tensor_mul(
                        eT_v[:, :, qe : qe + P],
                        eT_v[:, :, qe : qe + P],
                        bmask[:, 2 * m : 2 * m + 2, :],
                    )
                nc.tensor.matmul(
                    pat[:, qe:512],
                    vo[:, j0, :],
                    eT[:, qe:512],
                    start=(idx == 0),
                    stop=False,
                )
                nc.tensor.matmul(
                    pat[:, qe:512],
                    vo[:, j1, :],
                    eT[:, 512 + qe : 1024],
                    start=False,
                    stop=(idx == npair - 1),
                )
            osb = sb_exp.tile([D + 1, 512], F32, tag="osb", name=f"osb{c}")
            nc.vector.tensor_copy(osb[:], pat[:])
            nc.sync.dma_start(outT[:, c * 512 : (c + 1) * 512], osb[:])

        # emission order = scheduler priority: each chunk pair's KV
        # projection + Q projection + transposes are emitted BEFORE the
        # previous chunk's phase C so the next chunk's inputs are ready
        # the moment the ACT pipeline drains
        xk_tiles = {}

        def mk(fn, *args):
            return lambda: fn(*args)

        def emit_b(kc):
            xk_tiles[kc] = phase_b(kc)

        def emit_q(c):
            phase_b_q(c, xk_tiles[2 * c], xk_tiles[2 * c + 1])

        def phase_c0_piece(pat0, m, h):
            # chunk-0 pair m restricted to query cols [a, b)
            qe = 128 * m
            a, b = max(qe, 256 * h), 256 * h + 256
            w = b - a
            if w <= 0:
                return
            j0, j1 = 2 * m, 2 * m + 1
            psc = ps_sc.tile([P, 1024], F32, tag="sc", name=f"s0_{m}_{h}")
            for ji, j in ((0, j0), (1, j1)):
                nc.tensor.matmul(
                    psc[:, 512 * ji : 512 * ji + w],
                    kv_all[0:D, j * P : (j + 1) * P],
                    qts[:, a:b],
                    start=True,
                    stop=True,
                )
            eT = sb_exp.tile([P, 1024], F32R, tag="eT", name=f"e0_{m}_{h}")
            psc_v = psc[:].rearrange("p (two x) -> p two x", x=512)
            eT_v = eT[:].rearrange("p (two x) -> p two x", x=512)
            nc.scalar.activation(
                eT_v[:, :, 0:w], psc_v[:, :, 0:w], AF.Exp, scale=0.125
            )
            if a <= qe < b:
                lo = qe - a
                nc.vector.tensor_mul(
                    eT_v[:, :, lo : lo + P],
                    eT_v[:, :, lo : lo + P],
                    bmask[:, 2 * m : 2 * m + 2, :],
                )
            nc.tensor.matmul(
                pat0[:, a:b], vo[:, j0, :], eT[:, 0:w],
                start=(m == 0), stop=False,
            )
            nc.tensor.matmul(
                pat0[:, a:b], vo[:, j1, :], eT[:, 512 : 512 + w],
                start=False, stop=(m == 3 and h == 1),
            )

        import os as _os
        C0_SPLIT = _os.environ.get("K_C0_SPLIT", "1") == "1"
        if C0_SPLIT:
            emit_b(0)
            build_masks()
            phase_b_q_half(0, xk_tiles[0], 0)
            phase_b_tr(0)
            pat0 = ps_at.tile([D + 1, 512], F32, tag="at", name="at0")
            phase_c0_piece(pat0, 0, 0)
            phase_c0_piece(pat0, 1, 0)
            emit_b(1)
            phase_b_q_half(0, xk_tiles[1], 1)
            phase_b_tr(1)
            phase_c0_piece(pat0, 0, 1)
            emit_b(2)
            phase_c0_piece(pat0, 1, 1)
            emit_b(3)
            phase_c0_piece(pat0, 2, 1)
            emit_q(1)
            phase_c0_piece(pat0, 3, 1)
            phase_b_tr(2)
            phase_b_tr(3)
            osb0 = sb_exp.tile([D + 1, 512], F32, tag="osb", name="osb0")
            nc.vector.tensor_copy(osb0[:], pat0[:])
            nc.sync.dma_start(outT[:, 0:512], osb0[:])
            c_start = 1
        else:
            emit_b(0)
            build_masks()
            emit_b(1)
            emit_q(0)
            phase_b_tr(0)
            phase_b_tr(1)
            c_start = 0
        inj_at = {0: [0, 1, 2, 3], 1: [2, 4, 6, 7], 2: [3, 6, 9, 10]}
        for c in range(c_start, QC):
            if c < QC - 1:
                cn = c + 1
                pts = inj_at[c]
                inject = {
                    pts[0]: [mk(emit_b, 2 * cn)],
                    pts[1]: [mk(emit_b, 2 * cn + 1)],
                    pts[2]: [mk(emit_q, cn)],
                    pts[3]: [mk(phase_b_tr, 2 * cn), mk(phase_b_tr, 2 * cn + 1)],
                }
            else:
                inject = None
            phase_c(c, inject)

    nc.compile()
    return nc


def _stage_inputs(x, Wq, bq, Wk, bk, Wv, bv):
    """Build the 8 per-core input dicts."""
    import ml_dtypes

    x = np.asarray(x, dtype=np.float32)
    w3 = np.concatenate(
        [np.asarray(Wq), np.asarray(Wk), np.asarray(Wv)], axis=1
    ).astype(ml_dtypes.bfloat16)
    bias2 = np.zeros((P, 2), dtype=np.float32)
    bias2[0:D, 0] = np.asarray(bk, dtype=np.float32)  # K rows 0:64
    bias2[0:D, 1] = np.asarray(bq, dtype=np.float32)  # Q rows 0:64
    # bv is applied on the host during the gather

    qv = np.arange(512)
    in_maps = []
    for core in range(N_CORES):
        b, h = divmod(core, 2)
        g = np.arange(NT)
        if h == 1:
            g = g ^ 1  # pair-swap so query tiles land on even slots
        xb = x[b].reshape(NT, P, E)[g]  # [32,128,512]
        xkT_c = np.ascontiguousarray(
            xb.reshape(S, E).T.astype(ml_dtypes.bfloat16)
        )  # [512, 4096] bf16
        qpos = (P * (2 * (qv // P) + h) + (qv % P)).astype(np.float32)
        qband = np.ascontiguousarray(np.broadcast_to(qpos, (P, 512)))
        kk = np.arange(P)
        jl = np.arange(8)
        kband = (P * (jl[None, :] ^ h) + kk[:, None]).astype(np.float32)
        in_maps.append(
            {
                "xkT": xkT_c,
                "w3": w3,
                "bias2": bias2,
                "qband": qband,
                "kband": np.ascontiguousarray(kband),
            }
        )
    return in_maps


def _gather_output(results, bv):
    """Merge 8 per-core outT [65, 2048] into the full [B, S, D] output."""
    out = np.empty((B, S, D), dtype=np.float32)
    bv = np.asarray(bv, dtype=np.float32)
    tg = np.array([8 * c + 2 * si for c in range(QC) for si in range(4)])
    for core in range(N_CORES):
        b, h = divmod(core, 2)
        ot = results[core]["outT"]  # [65, 2048]
        attn = ot[0:D] / ot[D : D + 1] + bv[:, None]  # denom + V bias
        blocks = attn.T.reshape(16, P, D)  # [(c,si), r, d]
        out.reshape(B, NT, P, D)[b, tg + h] = blocks
    return out


def kernel(x, Wq, bq, Wk, bk, Wv, bv):
    if "nc" not in _CACHE:
        _CACHE["nc"] = _build()
    nc = _CACHE["nc"]
    in_maps = _stage_inputs(x, Wq, bq, Wk, bk, Wv, bv)
    res = run_bass_kernel_spmd(nc, in_maps, core_ids=list(range(N_CORES)))
    return _gather_output(res.results, bv)
